# revision 7
# baseline (speedup 1.0000x reference)
"""ComplexOscillator Trainium2 kernel (8-core SPMD, full-I/O contract).

kernel(frequencies[16,64,96000] f32, initial_phase[16,64,1] f32) -> cos phases.

Sharding: batch*oscillator rows (1024) across 8 cores -> 128 rows/core = one
SBUF partition per row; the time axis (the cumsum axis) stays whole per core.

Per chunk of TC samples, ONE fused custom DVE op does the whole recurrence:
    v[k] = (f[k] < NYQ) * (f[k] * S16)       # anti-alias mask + scale to units
    y[k] = carry + v[0] + ... + v[k]          # inclusive fp32 cumsum (scan)
    u[k] = y[k] - ((y[k] + MAGIC) - MAGIC)    # wrap to [-8, 8] units
at ~1 cycle/column on the DVE (vs ~4.4 cycles for the unfused 4-instruction
sequence).  Phase is tracked in units of 1/16 turn (one turn = 16 units) and
SHIFTED by -4 units so that the final cosine needs no abs():
    cos(2*pi*y_true/16) = cos(pi*(y'+4)/8) = -sin(pi*u/8) = sin(-pi/8 * u)
with u in [-8, 8] the Sin LUT argument -pi/8*u stays inside its accurate
[-pi, pi] range.  The wrap's magic-number trick: (x + 1.5*2^27) - 1.5*2^27
rounds x to the nearest multiple of 16 (ulp at that magnitude is exactly 16),
u = x - k is Sterbenz-exact, and the wrapped carry u[:, -1] chains chunks
losslessly (mod one turn), bounding fp32 accumulation noise to the same
envelope as the fp32 reference.

Output is written bf16 (halves output HBM traffic; cos in [-1,1] keeps
bf16 quantization ~2e-3 rel) and upcast to f32 on the host.
"""

import numpy as np
import sys
import os

if "/opt/trn_rl_repo" not in sys.path:
    sys.path.insert(0, "/opt/trn_rl_repo")

import concourse.bass as bass
import concourse.bacc as bacc
import concourse.mybir as mybir
from concourse.tile import TileContext
from concourse.bass_utils import run_bass_kernel_spmd

P = 128
B, N, T = 16, 64, 96000
NCORES = 8
ROWS = B * N  # 1024
TC = int(os.environ.get("OSC_TC", "2000"))

# One turn (2*pi of phase) == 16 "units"; v = f * S16 units per sample.
S16 = float(np.float32(16.0 / 48000.0))
NYQ = 24000.0
U0_SCALE = float(np.float32(16.0 / (2.0 * np.pi)))  # phi (rad) -> units
MAGIC = 201326592.0  # 1.5*2^27: ulp 16 = one turn for x in [-2^26, 2^26]
NEG_PI_8 = float(np.float32(-np.pi / 8.0))

LAST_EXEC_NS = None
LAST_RESULTS = None


# --- fused custom DVE op: mask+scale+cumsum+wrap in one instruction -------
def _register_osc_op():
    import concourse.dve_ops as dve_ops_mod
    from concourse.dve_ops import DveOp
    from concourse.dve_spec import (
        Spec, lower, Src0, C0, C1, C2, C3, AluOp, Scan, _spill_c3_to_src1,
        _has_src1,
    )
    from concourse.dve_uop import DveOpSpec

    if "OSC_SCAN_ANT" in dve_ops_mod._SUB_OPCODE_FOR_NAME:
        for op in dve_ops_mod.OPS:
            if op.name == "OSC_SCAN_ANT":
                return op

    def _osc_ref(in0, in1, s0, s1, imm2):
        f32 = np.float32
        Pp = in0.shape[0]
        f = in0.astype(f32).reshape(Pp, -1)
        scale = np.asarray(in1, f32).reshape(Pp, 1)
        carry = np.asarray(s0, f32).reshape(Pp, 1)
        g = (f * scale).astype(f32)
        v = ((f < f32(s1)).astype(f32) * g).astype(f32)
        y = np.empty_like(v)
        state = carry[:, 0].copy()
        for k in range(v.shape[1]):
            state = (state + v[:, k]).astype(f32)
            y[:, k] = state
        w = (y + f32(imm2)).astype(f32)
        kk = (w - f32(imm2)).astype(f32)
        return (y - kk).astype(f32)

    _v = (Src0 < C1) * (Src0 * C3)
    _y = Scan(AluOp.ADD, _v, init=C0)
    _body = _spill_c3_to_src1(_y - ((_y + C2) - C2))

    op = DveOp(
        "OSC_SCAN_ANT",
        Spec(body=_body, reference=_osc_ref),
        subdim=False,
        uops_sha={},
    )
    for _ver in ("v3",):
        _spec_l = DveOpSpec(
            name=op.name,
            opcode=None,
            uops=lower(op.spec, ver=_ver),
            rd1_en=_has_src1(op.spec),
        )
        op.uops_sha[_ver] = _spec_l.sha(_ver)

    dve_ops_mod.OPS.append(op)
    dve_ops_mod._SUB_OPCODE_FOR_NAME[op.name] = (
        max(dve_ops_mod._SUB_OPCODE_FOR_NAME.values()) + 1
    )
    assert dve_ops_mod._SUB_OPCODE_FOR_NAME[op.name] < 0x20
    dve_ops_mod.CUSTOM_DVE_SPECS[op.name] = op.spec
    return op


def _build(T=T, TC_DMA=None, TC_OP=None, fbufs=None, obufs=None, ubufs=4):
    TC_DMA = TC_DMA or int(os.environ.get("OSC_TC_DMA", "8000"))
    TC_OP = TC_OP or int(os.environ.get("OSC_TC_OP", "2000"))
    fbufs = fbufs or int(os.environ.get("OSC_FBUFS", "4"))
    obufs = obufs or int(os.environ.get("OSC_OBUFS", "2"))
    assert T % TC_DMA == 0 and TC_DMA % TC_OP == 0
    osc_op = _register_osc_op()
    nchunks = T // TC_DMA
    nsub = TC_DMA // TC_OP
    nc = bacc.Bacc()
    freq = nc.declare_dram_parameter("freq", [P, T], mybir.dt.float32, isOutput=False)
    ph0 = nc.declare_dram_parameter("ph0", [P, 1], mybir.dt.float32, isOutput=False)
    outd = nc.declare_dram_parameter("out", [P, T], mybir.dt.bfloat16, isOutput=True)

    with TileContext(nc) as tc:
        with (
            tc.tile_pool(name="const", bufs=1) as cpool,
            tc.tile_pool(name="fin", bufs=fbufs) as fpool,
            tc.tile_pool(name="u", bufs=ubufs) as upool,
            tc.tile_pool(name="o", bufs=obufs) as opool,
        ):
            # u0 = ph0 * (16/2pi) - 4 is precomputed on the host
            u0 = cpool.tile([P, 1], mybir.dt.float32)
            nc.sync.dma_start(out=u0[:], in_=ph0[:])
            s16 = cpool.tile([P, 1], mybir.dt.float32)
            nc.vector.memset(s16[:], S16)

            prev_u = None
            for j in range(nchunks):
                sl = slice(j * TC_DMA, (j + 1) * TC_DMA)
                f = fpool.tile([P, TC_DMA], mybir.dt.float32)
                if j == 0:
                    # slice-wise loads so the first op starts after TC_OP cols
                    for s in range(nsub):
                        ss = slice(s * TC_OP, (s + 1) * TC_OP)
                        nc.sync.dma_start(out=f[:, ss], in_=freq[:, ss])
                else:
                    nc.sync.dma_start(out=f[:], in_=freq[:, sl])

                o = opool.tile([P, TC_DMA], mybir.dt.bfloat16)
                for s in range(nsub):
                    ss = slice(s * TC_OP, (s + 1) * TC_OP)
                    u = upool.tile([P, TC_OP], mybir.dt.float32)
                    init = u0[:, 0:1] if prev_u is None else prev_u[:, TC_OP - 1 : TC_OP]
                    nc.vector._custom_dve(
                        osc_op, out=u[:], in0=f[:, ss], in1=s16[:],
                        s0=init, s1=NYQ, imm2=MAGIC,
                    )
                    prev_u = u
                    nc.scalar.activation(
                        o[:, ss], u[:], mybir.ActivationFunctionType.Sin,
                        bias=0.0, scale=NEG_PI_8,
                    )
                    if j == nchunks - 1:
                        # drain the tail early, slice by slice
                        nc.sync.dma_start(
                            out=outd[:, j * TC_DMA + s * TC_OP : j * TC_DMA + (s + 1) * TC_OP],
                            in_=o[:, ss],
                        )
                if j < nchunks - 1:
                    nc.sync.dma_start(out=outd[:, sl], in_=o[:])
    nc.compile()
    return nc


def kernel(frequencies: np.ndarray, initial_phase: np.ndarray) -> np.ndarray:
    global LAST_EXEC_NS, LAST_RESULTS
    f = np.ascontiguousarray(frequencies, dtype=np.float32).reshape(ROWS, T)
    p = np.ascontiguousarray(initial_phase, dtype=np.float32).reshape(ROWS, 1)
    # phase (rad) -> units of 1/16 turn, shifted by -4 (see _build docstring)
    p = (p * np.float32(U0_SCALE) - np.float32(4.0)).astype(np.float32)

    nc = _build()
    rows_per_core = ROWS // NCORES  # 128
    in_maps = []
    for c in range(NCORES):
        r0 = c * rows_per_core
        in_maps.append({
            "freq": f[r0 : r0 + rows_per_core],
            "ph0": p[r0 : r0 + rows_per_core],
        })

    trace = os.environ.get("OSC_TRACE", "0") == "1"
    res = run_bass_kernel_spmd(
        nc, in_maps, list(range(NCORES)), trace=trace,
    )
    LAST_EXEC_NS = res.exec_time_ns
    LAST_RESULTS = res
    out = np.empty((ROWS, T), dtype=np.float32)
    for c in range(NCORES):
        out[c * rows_per_core : (c + 1) * rows_per_core] = np.asarray(
            res.results[c]["out"], dtype=np.float32)
    return out.reshape(B, N, T)
